# revision 1
# baseline (speedup 1.0000x reference)
"""Trainium2 Bass kernel for 3D neighborhood attention (sparse_attention).

Problem: q,k [1,40,40,40,48] fp32, rpb [8,3,3,3]; out [1,24,40,40,40].
Per voxel x: logits[h,kk] = scale * <q[x,h,:], k[x+off_kk,h,:]> + rpb[h,kk]
(zero-padded k at boundaries, kk over 3x3x3 offsets), p = softmax over kk,
out[x,h,:] = sum_kk p[h,kk] * off_kk  (constant integer offsets as values).

Sharding: spatial-parallel over H (40 -> 8 slabs of 5). Each core gets its
q slab plus a host-side im2col of the 27 shifted k views for its slab
(halo handled on host), so on-core everything is token-parallel with
tokens on SBUF partitions (2 tokens per partition) and no cross-partition
data movement. The PV contraction exploits that the "values" are the
constant offsets in {-1,0,1}^3: out_i = (sum of exp over di=+1 block) -
(sum over di=-1 block), so it is pure block reductions, no multiplies.
"""

import numpy as np

import concourse.bass as bass
import concourse.tile as tile
from concourse import bacc, mybir
from concourse.bass_utils import run_bass_kernel_spmd

NH = 8
HD = 6
DIM = NH * HD
KS = 3
NT = KS**3  # 27
SCALE = HD**-0.5
H = W = T = 40
N_CORES = 8
SLAB = H // N_CORES          # 5 rows of H per core
TOK = SLAB * W * T           # 8000 tokens per core
P = 128
TPP = 2                      # tokens per partition
TILES = 32                   # ceil(8000 / 256)
TOKP = TILES * P * TPP       # 8192
FKC = NT * DIM               # 1296  (kk, c) free dim per token
FKH = NT * NH                # 216   (kk, h) free dim per token

_prog_cache = {}


def _build_program():
    fp32 = mybir.dt.float32
    nc = bacc.Bacc("TRN2", target_bir_lowering=False, debug=False,
                   num_devices=N_CORES)
    qs = nc.dram_tensor("qs", [TILES, P, TPP * DIM], fp32,
                        kind="ExternalInput").ap()
    kn = nc.dram_tensor("kn", [TILES, P, TPP * FKC], fp32,
                        kind="ExternalInput").ap()
    rpbt = nc.dram_tensor("rpbt", [P, FKH], fp32, kind="ExternalInput").ap()
    out = nc.dram_tensor("out", [TILES, P, TPP * 3 * NH], fp32,
                         kind="ExternalOutput").ap()

    X = mybir.AxisListType.X
    XY = mybir.AxisListType.XY
    ADD = mybir.AluOpType.add

    with tile.TileContext(nc) as tc:
        with (
            tc.tile_pool(name="consts", bufs=1) as cpool,
            tc.tile_pool(name="kin", bufs=3) as kpool,
            tc.tile_pool(name="qin", bufs=3) as qpool,
            tc.tile_pool(name="prod", bufs=2) as ppool,
            tc.tile_pool(name="logit", bufs=3) as lpool,
            tc.tile_pool(name="expv", bufs=3) as epool,
            tc.tile_pool(name="small", bufs=16) as spool,
            tc.tile_pool(name="outp", bufs=3) as opool,
        ):
            rpb_sb = cpool.tile([P, FKH], fp32)
            nc.sync.dma_start(rpb_sb[:], rpbt[:])

            for ti in range(TILES):
                kt = kpool.tile([P, TPP * FKC], fp32)
                nc.sync.dma_start(kt[:], kn[ti])
                qt = qpool.tile([P, TPP * DIM], fp32)
                nc.sync.dma_start(qt[:], qs[ti])

                # P4[p, j, kk, c] = kn[p, j, kk, c] * q[p, j, c]
                pt = ppool.tile([P, TPP * FKC], fp32)
                q_b = (qt[:].rearrange("p (j c) -> p j c", j=TPP)
                       .unsqueeze(2).broadcast_to([P, TPP, NT, DIM]))
                nc.vector.tensor_mul(
                    pt[:].rearrange("p (j kk c) -> p j kk c", j=TPP, kk=NT),
                    kt[:].rearrange("p (j kk c) -> p j kk c", j=TPP, kk=NT),
                    q_b,
                )
                # L[p, (j,kk,h)] = sum_d P4[p, j, (kk,h), d]
                lt = lpool.tile([P, TPP * FKH], fp32)
                nc.vector.tensor_reduce(
                    lt[:],
                    pt[:].rearrange("p (j kh d) -> p j kh d", j=TPP, d=HD),
                    axis=X, op=ADD,
                )
                # L2 = L + rpb  (q was pre-scaled by SCALE on host)
                l2 = lpool.tile([P, TPP * FKH], fp32)
                rpb_b = rpb_sb[:].unsqueeze(1).broadcast_to([P, TPP, FKH])
                nc.vector.tensor_add(
                    l2[:].rearrange("p (j f) -> p j f", j=TPP),
                    lt[:].rearrange("p (j f) -> p j f", j=TPP),
                    rpb_b,
                )
                # E = exp(L2)  (ScalarE, overlaps with DVE)
                et = epool.tile([P, TPP * FKH], fp32)
                nc.scalar.activation(et[:], l2[:],
                                     mybir.ActivationFunctionType.Exp)

                # Softmax denominator: S0[p, (j,h)] = sum_kk E
                e_khk = et[:].rearrange("p (j kk h) -> p j h kk",
                                        j=TPP, kk=NT, h=NH)
                s0 = spool.tile([P, TPP * NH], fp32)
                nc.vector.tensor_reduce(s0[:], e_khk, axis=X, op=ADD)

                # Directional numerators via paired block sums over the
                # +-1 slabs of each axis (values are +-1/0).
                # E free layout: (j, di, dj, dl, h).  V layout: (o, j, pm, h)
                v_di = et[:].rearrange(
                    "p (j di dj dl h) -> p j di h (dj dl)",
                    j=TPP, di=KS, dj=KS, dl=KS, h=NH)
                v_dj = et[:].rearrange(
                    "p (j di dj dl h) -> p j dj h di dl",
                    j=TPP, di=KS, dj=KS, dl=KS, h=NH)
                v_dl = et[:].rearrange(
                    "p (j di dj dl h) -> p j dl h di dj",
                    j=TPP, di=KS, dj=KS, dl=KS, h=NH)

                vt = spool.tile([P, 3 * 2 * TPP * NH], fp32)  # [128, 96]
                npm = TPP * NH
                for o, (v, ax) in enumerate(((v_di, X), (v_dj, XY),
                                             (v_dl, XY))):
                    for pm in range(2):
                        nc.vector.tensor_reduce(
                            vt[:, (o * 2 + pm) * npm:(o * 2 + pm + 1) * npm],
                            v[:, :, 2 * pm], axis=ax, op=ADD)

                # S3[p, (o,j,h)] = V[.., pm=1] - V[.., pm=0]
                v5 = vt[:].rearrange("p (o pm j h) -> p o pm j h",
                                     o=3, pm=2, j=TPP)
                s3 = spool.tile([P, 3 * TPP * NH], fp32)
                nc.vector.tensor_sub(
                    s3[:].rearrange("p (o j h) -> p o j h", o=3, j=TPP),
                    v5[:, :, 1], v5[:, :, 0])

                rt = spool.tile([P, TPP * NH], fp32)
                nc.vector.reciprocal(rt[:], s0[:])
                # out[p, (o,j,h)] = S3 * (1/S0)
                ot = opool.tile([P, TPP * 3 * NH], fp32)
                r_b = (rt[:].rearrange("p (j h) -> p j h", j=TPP)
                       .unsqueeze(1).broadcast_to([P, 3, TPP, NH]))
                nc.vector.tensor_mul(
                    ot[:].rearrange("p (o j h) -> p o j h", o=3, j=TPP),
                    s3[:].rearrange("p (o j h) -> p o j h", o=3, j=TPP),
                    r_b)
                nc.sync.dma_start(out[ti], ot[:])

    nc.compile()
    return nc


def _host_prep(q, k, rpb):
    q = np.asarray(q, dtype=np.float32)
    k = np.asarray(k, dtype=np.float32)
    rpb = np.asarray(rpb, dtype=np.float32)

    q0 = (q[0] * SCALE).astype(np.float32)          # [40,40,40,48]
    kp = np.pad(k[0], ((1, 1), (1, 1), (1, 1), (0, 0)))  # [42,42,42,48]
    win = np.lib.stride_tricks.sliding_window_view(kp, (KS, KS, KS),
                                                   axis=(0, 1, 2))
    # win: [40,40,40,48,3,3,3] -> [40,40,40,(kk,c)]
    win = np.ascontiguousarray(win.transpose(0, 1, 2, 4, 5, 6, 3))
    win = win.reshape(H, W, T, FKC)

    rpb_kh = np.ascontiguousarray(rpb.reshape(NH, NT).T).reshape(FKH)
    rpb_t = np.broadcast_to(rpb_kh, (P, FKH)).copy()

    in_maps = []
    for i in range(N_CORES):
        h0 = i * SLAB
        q_pad = np.zeros((TOKP, DIM), np.float32)
        q_pad[:TOK] = q0[h0:h0 + SLAB].reshape(TOK, DIM)
        kn_pad = np.zeros((TOKP, FKC), np.float32)
        kn_pad[:TOK] = win[h0:h0 + SLAB].reshape(TOK, FKC)
        in_maps.append({
            "qs": q_pad.reshape(TILES, P, TPP * DIM),
            "kn": kn_pad.reshape(TILES, P, TPP * FKC),
            "rpbt": rpb_t,
        })
    return in_maps


def _assemble(results):
    slabs = []
    for i in range(N_CORES):
        o = results[i]["out"].reshape(TILES, P, 3, TPP, NH)
        o = o.transpose(0, 1, 3, 2, 4).reshape(TOKP, 3, NH)[:TOK]
        o = o.reshape(SLAB, W, T, 3, NH)
        # channel order in reference: c = h*3 + o
        slabs.append(o.transpose(0, 1, 2, 4, 3).reshape(SLAB, W, T, 3 * NH))
    full = np.concatenate(slabs, axis=0)             # [40,40,40,24]
    return np.ascontiguousarray(full.transpose(3, 0, 1, 2))[None]


def _run(q, k, rpb, **spmd_kwargs):
    if "prog" not in _prog_cache:
        _prog_cache["prog"] = _build_program()
    nc = _prog_cache["prog"]
    in_maps = _host_prep(q, k, rpb)
    res = run_bass_kernel_spmd(nc, in_maps, list(range(N_CORES)),
                               **spmd_kwargs)
    return _assemble(res.results), res


def kernel(q, k, rpb):
    out, _ = _run(q, k, rpb)
    return out



# revision 6
# speedup vs baseline: 3.2266x; 3.2266x over previous
"""Trainium2 Bass kernel for 3D neighborhood attention (sparse_attention).

Problem: q,k [1,40,40,40,48] fp32, rpb [8,3,3,3]; out [1,24,40,40,40].
Per voxel x: logits[h,kk] = scale * <q[x,h,:], k[x+off_kk,h,:]> + rpb[h,kk]
(zero-padded k at boundaries, kk over 3x3x3 offsets), p = softmax over kk,
out[x,h,:] = sum_kk p[h,kk] * off_kk  (constant integer offsets as values).

Sharding: spatial over H (40 -> 8 slabs of 5 rows per core).

Per-core dataflow (all engines busy):
 - partitions = (head h in 8) x (chunk in 16), chunks tile (W,T) into 4x4
   blocks of 10x10; each partition owns a 5x10x10 = 500-token interior plus
   a zero-padded 7x12x12 halo'd copy of k (host-prepared, fp16).
 - DVE: 54 fp16 tensor_mul (2x mode) form all 27 shifted q*k products
   (free-axis window shifts; d-major layout [d,kk-slot,x]).
 - PE pass 1: the d-reduction is a transpose-with-accumulation: for each
   (kk, 128-token x-run), 6 matmuls (stationary = product slab, moving =
   fp16 identity) accumulate sum_d prod into PSUM transposed as
   [x-run, (h,chunk)], plus a 7th matmul adding rpb[h,kk] (a host-built
   constant slab).  fp32 PSUM accumulation = exact d-sum.
 - ACT: exp evacuates PSUM -> SBUF e^T tiles (one [128,512] instr per kk).
 - PE pass 2: softmax denominator Z and the three numerators (the values
   are the constant offsets in {-1,0,1}^3, and exp(l+rpb) sums are plain
   +/- accumulations) via matmuls with stationary = e^T tile and moving =
   +I / -I, accumulated over kk in PSUM [(h,chunk), (4,x-run)].
 - DVE: reciprocal_approx_fast + 3 multiplies -> out[h,chunk,(o,x)] fp16.
"""

import numpy as np

import concourse.bass as bass
import concourse.tile as tile
from concourse import bacc, mybir
from concourse.bass_utils import run_bass_kernel_spmd

NH = 8
HD = 6
DIM = NH * HD
KS = 3
NT = KS**3  # 27
SCALE = HD**-0.5
H = W = T = 40
N_CORES = 8
SLAB = H // N_CORES          # 5 rows of H per core
P = 128

CA, CB, CC = SLAB, 10, 10    # chunk interior dims (h-rows, w, t)
KA, KB, KC = CA + 2, CB + 2, CC + 2   # halo'd k block dims (7, 12, 12)
NCW, NCT = W // CB, T // CC  # 4 x 4 chunk grid
NCH = NCW * NCT              # 16 chunks -> 8 heads * 16 chunks = 128
X = CA * CB * CC             # 500 interior tokens per chunk
KX = KA * KB * KC            # 1008 halo'd tokens
# four 128-token x-runs covering the 500-token interior (last overlaps)
XRUNS = [(0, 128), (128, 256), (256, 384), (X - P, X)]

_prog_cache = {}


def _win_ap(kv, d, oi, oj, t):
    """Shifted-window AP on the halo'd k tile kv [128, 6, 7, 12, 12]:
    [p, a(5), b(10), c(10)] with element = kv[p, d, 1+oi+a, 1+oj+b, t+c]
    (t in 0..2 is the T-shift slot; the ISA allows only 3 free dims)."""
    base = kv[:, d, 1 + oi, 1 + oj, t]
    part = kv.ap[0]  # [partition_stride, 128]
    return bass.AP(
        base.tensor,
        base.offset,
        [part, [KB * KC, CA], [KC, CB], [1, CC]],
    )


def _build_program():
    fp32 = mybir.dt.float32
    fp16 = mybir.dt.float16
    nc = bacc.Bacc("TRN2", target_bir_lowering=False, debug=False,
                   num_devices=N_CORES)
    qv = nc.dram_tensor("qv", [P, HD, X], fp16, kind="ExternalInput").ap()
    kv = nc.dram_tensor("kv", [P, HD, KA, KB, KC], fp16,
                        kind="ExternalInput").ap()
    rpbs = nc.dram_tensor("rpbs", [P, NT, P], fp16, kind="ExternalInput").ap()
    ident = nc.dram_tensor("ident", [P, 2, P], fp16,
                           kind="ExternalInput").ap()  # [+I | -I]
    out = nc.dram_tensor("out", [P, 3, X], fp16, kind="ExternalOutput").ap()

    EXP = mybir.ActivationFunctionType.Exp

    with tile.TileContext(nc) as tc:
        with (
            tc.tile_pool(name="consts", bufs=1) as cpool,
            tc.tile_pool(name="prod", bufs=2) as ppool,
            tc.tile_pool(name="ev", bufs=1) as epool,
            tc.tile_pool(name="rec", bufs=2) as rpool,
            tc.tile_pool(name="outp", bufs=1) as opool,
            tc.psum_pool(name="ps1", bufs=4) as ps1pool,
            tc.psum_pool(name="ps2", bufs=2) as ps2pool,
        ):
            qv_sb = cpool.tile([P, HD, X], fp16)
            nc.sync.dma_start(qv_sb[:], qv[:])
            kv_sb = cpool.tile([P, HD, KA, KB, KC], fp16)
            nc.sync.dma_start(kv_sb[:], kv[:])
            rpb_sb = cpool.tile([P, NT, P], fp16)
            nc.sync.dma_start(rpb_sb[:], rpbs[:])
            id_sb = cpool.tile([P, 2, P], fp16)
            nc.sync.dma_start(id_sb[:], ident[:])

            e_sb = epool.tile([P, NT, 4, P], fp16)  # e^T: [x, (kk, xrun, hc)]
            out_sb = opool.tile([P, 3, X], fp16)

            # ---- Phase A: products (DVE), d-sum transposes (PE), exp (ACT)
            for oi in (-1, 0, 1):
                for oj in (-1, 0, 1):
                    prod = ppool.tile([P, HD, KS, X], fp16)
                    pv = prod[:].rearrange("p d t (a b c) -> p d t a b c",
                                           a=CA, b=CB, c=CC)
                    for d in range(HD):
                        q_b = qv_sb[:, d].rearrange("p (a b c) -> p a b c",
                                                    a=CA, b=CB)
                        for t in range(KS):
                            nc.vector.tensor_mul(
                                pv[:, d, t], _win_ap(kv_sb[:], d, oi, oj, t),
                                q_b)
                    for t in range(KS):
                        kk = (oi + 1) * 9 + (oj + 1) * 3 + t
                        ps1 = ps1pool.tile([P, 4, P], fp32)
                        for xb, (x0, x1) in enumerate(XRUNS):
                            for d in range(HD):
                                nc.tensor.matmul(
                                    ps1[:, xb], prod[:, d, t, x0:x1],
                                    id_sb[:, 0], start=(d == 0), stop=False)
                            nc.tensor.matmul(
                                ps1[:, xb], rpb_sb[:, kk, 0:P],
                                id_sb[:, 0], start=False, stop=True)
                        nc.scalar.activation(
                            e_sb[:, kk].rearrange("p r c -> p (r c)"),
                            ps1[:].rearrange("p r c -> p (r c)"), EXP)

            # ---- Phase B: softmax-weighted sums (PE) + divide (DVE)
            # accumulation slot o: 0 = Z, 1..3 = numerators for (i, j, l)
            def _coef(kk, o):
                return (1, kk // 9 - 1, (kk // 3) % 3 - 1, kk % 3 - 1)[o]

            for xb, (x0, x1) in enumerate(XRUNS):
                ps2 = ps2pool.tile([P, 4, P], fp32)
                # one accumulation region at a time: interleaved start/stop
                # groups across psum regions give wrong results on HW
                for o in range(4):
                    used = [kk for kk in range(NT) if _coef(kk, o)]
                    for n, kk in enumerate(used):
                        nc.tensor.matmul(
                            ps2[:, o], e_sb[:, kk, xb],
                            id_sb[:, 0 if _coef(kk, o) > 0 else 1],
                            start=(n == 0), stop=(n == len(used) - 1))
                rr = rpool.tile([P, P], fp32)
                nc.vector.reciprocal_approx_fast(rr[:], ps2[:, 0])
                lo = 0 if xb < 3 else 384 - x0  # skip overlap with run 2
                r_b = (rr[:, lo:].unsqueeze(1)
                       .broadcast_to([P, 3, P - lo]))
                nc.vector.tensor_mul(out_sb[:, :, x0 + lo:x1],
                                     ps2[:, 1:4, lo:], r_b)

            nc.sync.dma_start(out[:], out_sb[:])

    nc.compile()
    return nc


def _host_prep(q, k, rpb):
    q0 = (np.asarray(q[0], np.float32) * SCALE)
    kp = np.pad(np.asarray(k[0], np.float32),
                ((1, 1), (1, 1), (1, 1), (0, 0)))
    rpb_f = np.asarray(rpb, np.float32).reshape(NH, NT)

    # rpbs[p=(h,ch), kk, x] = rpb[h, kk]
    rpbs = np.broadcast_to(
        rpb_f[:, None, :, None], (NH, NCH, NT, P)).reshape(P, NT, P)
    rpbs = np.ascontiguousarray(rpbs, dtype=np.float16)
    ident = np.stack([np.eye(P, dtype=np.float16),
                      -np.eye(P, dtype=np.float16)], axis=1)  # [P, 2, P]
    ident = np.ascontiguousarray(ident)

    in_maps = []
    for i in range(N_CORES):
        h0 = i * SLAB
        # qv[p=(h, wB*4+tB), d, (a,b,c)]
        qs = q0[h0:h0 + SLAB].reshape(CA, NCW, CB, NCT, CC, NH, HD)
        qv = qs.transpose(5, 1, 3, 6, 0, 2, 4).reshape(P, HD, X)
        # kv[p, d, A, B, C] halo'd (kp index = global + 1)
        ks = kp[h0:h0 + KA]  # [7, 42, 42, 48]
        kv = np.empty((NH, NCW, NCT, HD, KA, KB, KC), np.float32)
        for wb in range(NCW):
            for tb in range(NCT):
                blk = ks[:, 10 * wb:10 * wb + KB, 10 * tb:10 * tb + KC]
                kv[:, wb, tb] = blk.reshape(KA, KB, KC, NH, HD).transpose(
                    3, 4, 0, 1, 2)
        kv = kv.reshape(P, HD, KA, KB, KC)
        in_maps.append({
            "qv": np.ascontiguousarray(qv, dtype=np.float16),
            "kv": np.ascontiguousarray(kv, dtype=np.float16),
            "rpbs": rpbs,
            "ident": ident,
        })
    return in_maps


def _assemble(results):
    full = np.empty((NH, 3, H, W, T), np.float32)
    for i in range(N_CORES):
        o = np.asarray(results[i]["out"], np.float32)
        o = o.reshape(NH, NCW, NCT, 3, CA, CB, CC)
        # -> [h, o, a, wB, b, tB, c]
        o = o.transpose(0, 3, 4, 1, 5, 2, 6).reshape(NH, 3, CA, W, T)
        full[:, :, i * SLAB:(i + 1) * SLAB] = o
    return full.reshape(NH * 3, H, W, T)[None]


def _run(q, k, rpb, **spmd_kwargs):
    if "prog" not in _prog_cache:
        _prog_cache["prog"] = _build_program()
    nc = _prog_cache["prog"]
    in_maps = _host_prep(q, k, rpb)
    res = run_bass_kernel_spmd(nc, in_maps, list(range(N_CORES)),
                               **spmd_kwargs)
    return _assemble(res.results), res


def kernel(q, k, rpb):
    out, _ = _run(q, k, rpb)
    return out
